# revision 11
# baseline (speedup 1.0000x reference)
"""Deformable RoI pooling (deform_psroi_pooling, group_size=1) on 8 Trainium2
NeuronCores via Bass/Tile.

Strategy
--------
Per roi r and output bin (ph, pw) the reference computes a weighted sum of
bilinear taps over feature-map cells; folding bilinear weights, validity
masking and 1/cnt normalisation gives a per-roi sparse matmul

    out[r, :, bin] = sum_{cells q} S_r[q, bin] * F[b_r, :, q]

Device work per core (SPMD, one program, 8 cores):
  * feature map shipped channel-last [200, 304, 256] bf16,
  * rois dealt to the 4 cores of their image; rank v gets a canonical
    fetch rectangle (nh_v, nw_v) with nw_v in {8,16,32,64} and
    nh_v*nw_v a multiple of 128, the max over cores at that rank,
  * each rectangle is ONE direct HWDGE DMA whose DRAM origin is loaded
    from SBUF into a register at runtime (per-core dynamic, shapes
    static) -- no SWDGE descriptor generation (the Pool DSP emits
    descriptors at only ~8 ns each, which capped gather designs),
  * rectangle cells land row-major on partitions: cell i -> partition
    i%128, free chunk i//128, so every 128-cell slot belongs to one roi
    and one K=128 bf16 matmul per slot (lhsT = per-slot S columns [128,49])
    accumulates into the roi's [49, 256] PSUM tile,
  * PSUM -> SBUF bf16 downcast (scalar/vector alternating) -> HBM out
    [49, K*256] bf16; host upcasts.

bf16 data/weights halve HBM traffic and run the PE at 1 cycle/row
(fp32 is 4); PSUM accumulation stays fp32, keeping global rel err ~5e-3.
"""

import numpy as np

P = 7          # pooled size (== part size)
SPP = 4        # samples per part
SPATIAL_SCALE = np.float32(0.0625)
TRANS_STD = np.float32(0.1)
N_IMG, C_FEAT, H_FEAT, W_FEAT = 2, 256, 200, 304
NBINS = P * P                             # 49
N_CORES = 8

_f32 = np.float32


def _host_tables(rois: np.ndarray, offset: np.ndarray):
    """Mirror the reference position math bit-exactly in float32; per roi
    return (image, taps=(cells, bins, weights), bbox=(hmin, hmax, xmin,
    xmax))."""
    R = rois.shape[0]
    rois = rois.astype(np.float32, copy=False)
    offset = offset.astype(np.float32, copy=False)

    b = rois[:, 0].astype(np.int32)
    roi_start_w = np.round(rois[:, 1]) * SPATIAL_SCALE - _f32(0.5)
    roi_start_h = np.round(rois[:, 2]) * SPATIAL_SCALE - _f32(0.5)
    roi_end_w = (np.round(rois[:, 3]) + _f32(1.0)) * SPATIAL_SCALE - _f32(0.5)
    roi_end_h = (np.round(rois[:, 4]) + _f32(1.0)) * SPATIAL_SCALE - _f32(0.5)
    roi_w = np.maximum(roi_end_w - roi_start_w, _f32(0.1))
    roi_h = np.maximum(roi_end_h - roi_start_h, _f32(0.1))
    bin_w = roi_w / _f32(P)
    bin_h = roi_h / _f32(P)
    sub_w = bin_w / _f32(SPP)
    sub_h = bin_h / _f32(SPP)

    ph = np.arange(P, dtype=np.float32)
    pw = np.arange(P, dtype=np.float32)
    tx = offset[:, 0] * TRANS_STD                       # [R, P, P]
    ty = offset[:, 1] * TRANS_STD

    wstart = (pw[None, None, :] * bin_w[:, None, None]
              + roi_start_w[:, None, None] + tx * roi_w[:, None, None])
    hstart = (ph[None, :, None] * bin_h[:, None, None]
              + roi_start_h[:, None, None] + ty * roi_h[:, None, None])

    s = np.arange(SPP, dtype=np.float32)
    wpos = wstart[..., None, None] + s[None, None, None, None, :] * sub_w[:, None, None, None, None]
    hpos = hstart[..., None, None] + s[None, None, None, :, None] * sub_h[:, None, None, None, None]

    W = W_FEAT
    H = H_FEAT
    valid = ((wpos > _f32(-0.5)) & (wpos < _f32(W) - _f32(0.5))
             & (hpos > _f32(-0.5)) & (hpos < _f32(H) - _f32(0.5)))
    wc = np.clip(wpos, _f32(0.0), _f32(W - 1.0))
    hc = np.clip(hpos, _f32(0.0), _f32(H - 1.0))
    x0 = np.floor(wc)
    y0 = np.floor(hc)
    dx = wc - x0
    dy = hc - y0
    x0i = x0.astype(np.int32)
    y0i = y0.astype(np.int32)
    x1i = np.minimum(x0i + 1, W - 1)
    y1i = np.minimum(y0i + 1, H - 1)

    cnt = valid.sum(axis=(-1, -2)).astype(np.float32)           # [R, P, P]
    inv = _f32(1.0) / np.maximum(cnt, _f32(1.0))

    one = _f32(1.0)
    w00 = (one - dx) * (one - dy)
    w01 = dx * (one - dy)
    w10 = (one - dx) * dy
    w11 = dx * dy

    bins = np.broadcast_to(
        (np.arange(P)[:, None] * P + np.arange(P)[None, :])[None, :, :, None, None],
        valid.shape,
    )
    scale = np.broadcast_to(inv[:, :, :, None, None], valid.shape)

    per_roi = []
    for r in range(R):
        v = valid[r].ravel()
        if not v.any():
            per_roi.append((int(b[r]), None, (0, 0, 0, 0)))
            continue
        shp = valid[r].shape
        bc = lambda a: np.broadcast_to(a, shp).ravel()[v]
        sc = bc(scale[r]).astype(np.float32)
        bn = bc(bins[r]).astype(np.int64)
        cy0 = bc(y0i[r]).astype(np.int64)
        cy1 = bc(y1i[r]).astype(np.int64)
        cx0 = bc(x0i[r]).astype(np.int64)
        cx1 = bc(x1i[r]).astype(np.int64)
        ws = [bc(w00[r]) * sc, bc(w01[r]) * sc,
              bc(w10[r]) * sc, bc(w11[r]) * sc]
        hs = np.concatenate([cy0, cy0, cy1, cy1])
        xs = np.concatenate([cx0, cx1, cx0, cx1])
        w_all = np.concatenate(ws).astype(np.float64)
        bin_all = np.concatenate([bn] * 4)
        bbox = (int(hs.min()), int(hs.max()), int(xs.min()), int(xs.max()))
        per_roi.append((int(b[r]), (hs, xs, bin_all, w_all), bbox))
    return per_roi


def _rect_shape(bbox):
    """Per-roi fetch rectangle: nw class in {8,16,32,64}, nh padded so
    nh*nw is a multiple of 128."""
    hmin, hmax, xmin, xmax = bbox
    nh = hmax - hmin + 1
    nw = xmax - xmin + 1
    for nwc in (8, 16, 32, 64):
        if nw <= nwc:
            break
    m = 128 // nwc
    nhc = -(-nh // m) * m
    return nhc, nwc


def _deal_to_cores(per_roi):
    """Assign rois to cores (cores 0-3 image 0, 4-7 image 1) snake-dealt by
    descending rectangle area; canonical profile per rank v is the
    elementwise max rectangle over cores."""
    img_rois = {0: [], 1: []}
    for rid, (img, taps, bbox) in enumerate(per_roi):
        nh, nw = _rect_shape(bbox)
        img_rois[img].append((nh * nw, nw, nh, rid))
    core_rois = [[] for _ in range(N_CORES)]
    for img, lst in img_rois.items():
        lst.sort(reverse=True)
        cores = list(range(4 * img, 4 * img + 4))
        for i, item in enumerate(lst):
            k = i % 8
            c = cores[k] if k < 4 else cores[7 - k]
            core_rois[c].append(item)
    for c in range(N_CORES):
        core_rois[c].sort(reverse=True)
    K = max(1, max(len(cr) for cr in core_rois))
    prof = []
    for v in range(K):
        nwv = max((cr[v][1] if v < len(cr) else 8) for cr in core_rois)
        nhv = max((cr[v][2] if v < len(cr) else 16) for cr in core_rois)
        m = 128 // nwv
        nhv = -(-nhv // m) * m
        prof.append((nhv, nwv))
    return core_rois, tuple(prof)


_PROGRAM_CACHE: dict = {}


def _build_program(prof):
    key = prof
    if key in _PROGRAM_CACHE:
        return _PROGRAM_CACHE[key]

    from concourse import bass, mybir, bacc
    from concourse.expressions import make_scalar_value
    from concourse.tile import TileContext

    K = len(prof)
    kv = [nh * nw // 128 for nh, nw in prof]     # slots per rank
    base = np.concatenate([[0], np.cumsum(kv)]).astype(int)
    tot = int(base[-1])

    nc = bacc.Bacc("TRN2", target_bir_lowering=False, debug=False,
                   num_devices=N_CORES)
    dataS = nc.declare_dram_parameter("dataS", [H_FEAT * W_FEAT, C_FEAT],
                                      mybir.dt.bfloat16, isOutput=False)
    offc = nc.declare_dram_parameter("offc", [1, K],
                                     mybir.dt.int32, isOutput=False)
    spack = nc.declare_dram_parameter("spack", [128, tot * NBINS],
                                      mybir.dt.bfloat16, isOutput=False)
    out = nc.declare_dram_parameter("out", [NBINS, K * C_FEAT],
                                    mybir.dt.bfloat16, isOutput=True)

    with TileContext(nc) as tc:
        with (
            tc.tile_pool(name="const", bufs=1) as cpool,
            tc.tile_pool(name="gt", bufs=4) as gpool,
            tc.tile_pool(name="ps", bufs=6, space="PSUM") as pspool,
            tc.tile_pool(name="ob", bufs=4) as opool,
        ):
            off_t = cpool.tile([1, K], mybir.dt.int32)
            nc.sync.dma_start(out=off_t[:], in_=offc[:])
            s_t = cpool.tile([128, tot * NBINS], mybir.dt.bfloat16)
            scols = tot * NBINS
            nq = 16
            for q in range(nq):
                lo = q * scols // nq
                hi = (q + 1) * scols // nq
                if hi > lo:
                    nc.sync.dma_start(out=s_t[:, lo:hi], in_=spack[:, lo:hi])

            data_ap = dataS[:]
            for v in range(K):
                nh, nw = prof[v]
                k = kv[v]
                reg = nc.sync.alloc_register(f"ro{v}")
                nc.sync.reg_load(reg, off_t[0:1, v:v + 1])
                sv = make_scalar_value(reg, min_val=0,
                                       max_val=H_FEAT * W_FEAT - 1)
                mrows = 128 // nw            # grid rows per 128-cell slot
                gt = gpool.tile([128, k * C_FEAT], mybir.dt.bfloat16,
                                name="gt")
                for t in range(k):
                    src = bass.AP(
                        data_ap.tensor,
                        sv * C_FEAT + t * mrows * W_FEAT * C_FEAT,
                        [[W_FEAT * C_FEAT, mrows], [1, nw * C_FEAT]])
                    nc.sync.dma_start(
                        out=gt[:, t * C_FEAT:(t + 1) * C_FEAT], in_=src)
                ps = pspool.tile([NBINS, C_FEAT], mybir.dt.float32,
                                 name="ps")
                for t in range(k):
                    gs = int(base[v]) + t
                    nc.tensor.matmul(
                        ps[:],
                        lhsT=s_t[:, gs * NBINS:(gs + 1) * NBINS],
                        rhs=gt[:, t * C_FEAT:(t + 1) * C_FEAT],
                        start=(t == 0),
                        stop=(t == k - 1),
                    )
                ob = opool.tile([NBINS, C_FEAT], mybir.dt.bfloat16,
                                name="ob")
                if v % 2 == 0:
                    nc.scalar.copy(out=ob[:], in_=ps[:])
                else:
                    nc.vector.tensor_copy(out=ob[:], in_=ps[:])
                nc.sync.dma_start(out=out[:, v * C_FEAT:(v + 1) * C_FEAT],
                                  in_=ob[:])
    nc.compile()
    _PROGRAM_CACHE[key] = nc
    return nc


def _core_inputs(per_roi, core_rois, prof, dataS_imgs):
    import ml_dtypes

    K = len(prof)
    kv = [nh * nw // 128 for nh, nw in prof]
    base = np.concatenate([[0], np.cumsum(kv)]).astype(int)
    tot = int(base[-1])
    in_maps = []
    roi_of_v = []
    for c in range(N_CORES):
        img = 0 if c < 4 else 1
        offc = np.zeros((1, K), np.int32)
        sp = np.zeros((128, tot, NBINS), np.float32)
        rmap = [-1] * K
        for v, (_, _, _, rid) in enumerate(core_rois[c]):
            rmap[v] = rid
            _, taps, bbox = per_roi[rid]
            if taps is None:
                continue
            nh, nw = prof[v]
            hmin, hmax, xmin, xmax = bbox
            h0 = min(hmin, H_FEAT - nh)
            x0 = min(xmin, W_FEAT - nw)
            offc[0, v] = h0 * W_FEAT + x0
            hs, xs, bin_all, w_all = taps
            idx = (hs - h0) * nw + (xs - x0)            # cell in rectangle
            key = (int(base[v]) * 128 + idx) * NBINS + bin_all
            lo = int(base[v]) * 128 * NBINS
            S = np.bincount(key - lo, weights=w_all,
                            minlength=nh * nw * NBINS).astype(np.float32)
            S = S.reshape(nh * nw, NBINS)
            r = np.arange(nh * nw)
            sp[r % 128, int(base[v]) + r // 128, :] = S
        in_maps.append({
            "dataS": dataS_imgs[img],
            "offc": offc,
            "spack": sp.reshape(128, tot * NBINS).astype(ml_dtypes.bfloat16),
        })
        roi_of_v.append(rmap)
    return in_maps, roi_of_v


def _prepare(data, rois, offset):
    import ml_dtypes

    data = np.ascontiguousarray(data, dtype=np.float32)
    rois = np.asarray(rois, dtype=np.float32)
    offset = np.asarray(offset, dtype=np.float32)

    per_roi = _host_tables(rois, offset)
    core_rois, prof = _deal_to_cores(per_roi)
    nc = _build_program(prof)

    dataS_imgs = [
        np.ascontiguousarray(data[i].transpose(1, 2, 0)).reshape(
            H_FEAT * W_FEAT, C_FEAT).astype(ml_dtypes.bfloat16)
        for i in range(N_IMG)
    ]
    in_maps, roi_of_v = _core_inputs(per_roi, core_rois, prof, dataS_imgs)
    return nc, in_maps, roi_of_v, len(prof)


def _collect(results, roi_of_v, K, R):
    out_full = np.zeros((R, C_FEAT, P, P), np.float32)
    for c in range(N_CORES):
        o = np.asarray(results[c]["out"]).astype(np.float32)   # [49, K*256]
        o = o.reshape(NBINS, K, C_FEAT).transpose(1, 2, 0)     # [K, 256, 49]
        for v, rid in enumerate(roi_of_v[c]):
            if rid >= 0:
                out_full[rid] = o[v].reshape(C_FEAT, P, P)
    return out_full


def kernel(data: np.ndarray, rois: np.ndarray, offset: np.ndarray) -> np.ndarray:
    from concourse.bass_utils import run_bass_kernel_spmd

    R = rois.shape[0]
    nc, in_maps, roi_of_v, K = _prepare(data, rois, offset)
    res = run_bass_kernel_spmd(nc, in_maps, list(range(N_CORES)), trace=False)
    return _collect(res.results, roi_of_v, K, R)
